# revision 23
# baseline (speedup 1.0000x reference)
"""Trainium2 Bass kernel for nn_CasFlowSeqModel (VAE encoder + planar flows +
autoregressive GRU decoder).

Sharding: pure data-parallel over batch B=4096 across 8 NeuronCores
(512 rows/core), params replicated. On-chip layout is "transposed"
(feature-on-partitions, batch-on-free-dim) so the GRU recurrence needs no
transposes: state hT [RU=128 partitions, batch free] feeds matmuls directly.
GRU runs 513 fully-unrolled steps, 2 interleaved batch streams of 256 columns
to overlap PE / ACT(sigmoid+tanh) / DVE work. fp16 operands, fp32 PSUM.
"""

import sys

import numpy as np

sys.path.insert(0, "/opt/trn_rl_repo")

import concourse.bass as bass  # noqa: E402
from concourse import bacc  # noqa: E402
import concourse.mybir as mybir  # noqa: E402
import concourse.tile as tile  # noqa: E402
from concourse.bass_utils import run_bass_kernel_spmd  # noqa: E402
from concourse import bass_utils as _bu  # noqa: E402

# Re-enable walrus LDWEIGHTS scheduling opt (hardcoded off in bass_utils):
# overlaps weight loads with in-flight matmuls on the PE.
if not getattr(_bu, "_ldw_patched", False):
    _orig_run_command = _bu.run_command

    def _run_command_ldw(argv, **kw):
        argv = list(argv)
        return _orig_run_command(argv, **kw)

    _bu.run_command = _run_command_ldw
    _bu._ldw_patched = True

F16 = mybir.dt.float16
F32 = mybir.dt.float32
AF = mybir.ActivationFunctionType
OP = mybir.AluOpType

B, T = 4096, 512
NC_ = 8
BL = B // NC_          # 512 batch rows per core
HB = BL // 2           # 256 per stream
STEPS = T + 1          # 513
RU = 128
EMB = 128
ZD = 64
NF = 8
BERT = 768


def build_program(steps=STEPS):
    nc = bacc.Bacc("TRN2", target_bir_lowering=False)

    # ---- DRAM I/O (per core; names keyed in in_maps) ----
    d_bertT = nc.dram_tensor("bertT", [BERT, BL], F16, kind="ExternalInput")
    d_timesT = nc.dram_tensor("timesT", [T, BL], F32, kind="ExternalInput")
    d_epsT = nc.dram_tensor("epsT", [ZD, BL], F32, kind="ExternalInput")
    d_projT = nc.dram_tensor("projT", [BERT, EMB], F16, kind="ExternalInput")
    d_mlvT = nc.dram_tensor("mlvT", [EMB, 2 * ZD], F16, kind="ExternalInput")
    d_decT = nc.dram_tensor("decT", [ZD, RU], F32, kind="ExternalInput")
    d_WrT = nc.dram_tensor("WrT", [RU, RU], F16, kind="ExternalInput")
    d_WzT = nc.dram_tensor("WzT", [RU, RU], F16, kind="ExternalInput")
    d_WnT = nc.dram_tensor("WnT", [RU, RU], F16, kind="ExternalInput")
    d_L2r = nc.dram_tensor("L2r", [2, RU], F16, kind="ExternalInput")
    d_L2z = nc.dram_tensor("L2z", [2, RU], F16, kind="ExternalInput")
    d_winr = nc.dram_tensor("winr", [1, RU], F16, kind="ExternalInput")
    d_headT = nc.dram_tensor("headT", [RU, 2], F16, kind="ExternalInput")
    d_DT = nc.dram_tensor("DT", [128, 128], F32, kind="ExternalInput")
    d_EV = nc.dram_tensor("EV", [128, 1], F32, kind="ExternalInput")
    d_SC = nc.dram_tensor("SC", [128, 8], F32, kind="ExternalInput")
    d_FW = nc.dram_tensor("FW", [ZD, NF], F32, kind="ExternalInput")
    d_UH = nc.dram_tensor("UH", [1, NF * ZD], F32, kind="ExternalInput")
    d_MB = nc.dram_tensor("MB", [1, ZD], F32, kind="ExternalInput")
    d_FSC = nc.dram_tensor("FSC", [1, 3 * NF], F32, kind="ExternalInput")
    d_I64 = nc.dram_tensor("I64", [ZD, ZD], F32, kind="ExternalInput")

    d_outs = nc.dram_tensor("outs", [STEPS, 2, BL], F32, kind="ExternalOutput")
    d_misc = nc.dram_tensor("misc", [1, 4], F32, kind="ExternalOutput")

    with tile.TileContext(nc) as tc:
        with (
            tc.tile_pool(name="consts", bufs=1) as consts,
            tc.tile_pool(name="work", bufs=2) as work,
            tc.tile_pool(name="gwork", bufs=2) as gwork,
            tc.tile_pool(name="dram", bufs=1, space="DRAM") as dpool,
            tc.tile_pool(name="h0p", bufs=3) as h0pool,
            tc.tile_pool(name="h1p", bufs=3) as h1pool,
            tc.tile_pool(name="sw0", bufs=3) as sw0pool,
            tc.tile_pool(name="sw1", bufs=3) as sw1pool,
        ):
            d_xt = dpool.tile([STEPS, BL], F16, tag="xt")
            # ---------- constants to SBUF ----------
            def cload(dram_ap, shape, dtype, tag):
                t_ = consts.tile(shape, dtype, tag=tag)
                nc.sync.dma_start(out=t_, in_=dram_ap)
                return t_

            bertT_sb = cload(d_bertT.rearrange("(c p) b -> p c b", p=128),
                             [128, 6, BL], F16, "bertT")
            timesT_sb = cload(d_timesT.rearrange("(c p) b -> p c b", p=128),
                              [128, 4, BL], F32, "timesT")
            epsT_sb = cload(d_epsT[:, :], [ZD, BL], F32, "epsT")
            projT_sb = cload(d_projT.rearrange("(c p) m -> p c m", p=128),
                             [128, 6, EMB], F16, "projT")
            mlvT_sb = cload(d_mlvT[:, :], [EMB, 2 * ZD], F16, "mlvT")
            decT_sb = cload(d_decT[:, :], [ZD, RU], F32, "decT")
            WrT_sb = cload(d_WrT[:, :], [RU, RU], F16, "WrT")
            WzT_sb = cload(d_WzT[:, :], [RU, RU], F16, "WzT")
            WnT_sb = cload(d_WnT[:, :], [RU, RU], F16, "WnT")
            L2r_sb = cload(d_L2r[:, :], [2, RU], F16, "L2r")
            L2z_sb = cload(d_L2z[:, :], [2, RU], F16, "L2z")
            winr_sb = cload(d_winr[:, :], [1, RU], F16, "winr")
            headT_sb = cload(d_headT[:, :], [RU, 2], F16, "headT")
            DT_sb = cload(d_DT[:, :], [128, 128], F32, "DT")
            EV_sb = cload(d_EV[:, :], [128, 1], F32, "EV")
            SC_sb = cload(d_SC[:, :], [128, 8], F32, "SC")
            FW_sb = cload(d_FW[:, :], [ZD, NF], F32, "FW")
            UH_sb = cload(d_UH[:, :], [1, NF * ZD], F32, "UH")
            MB_sb = cload(d_MB[:, :], [1, ZD], F32, "MB")
            FSC_sb = cload(d_FSC[:, :], [1, 3 * NF], F32, "FSC")
            I64_sb = cload(d_I64[:, :], [ZD, ZD], F32, "I64")

            ones_c128 = consts.tile([128, 1], F16, tag="ones_c128")
            nc.vector.memset(ones_c128, 1.0)
            ones_c64 = consts.tile([ZD, 1], F32, tag="ones_c64")
            nc.vector.memset(ones_c64, 1.0)
            ones_r128 = consts.tile([1, 128], F16, tag="ones_r128")
            nc.vector.memset(ones_r128, 1.0)
            ones_r512 = consts.tile([1, BL], F32, tag="ones_r512")
            nc.vector.memset(ones_r512, 1.0)
            zrow16 = consts.tile([1, BL], F16, tag="zrow16")
            nc.vector.memset(zrow16, 0.0)
            # rotating x-staging tiles: row0 <- x_t via DMA, row1 = ones
            NSTAGE = 8
            stages = []
            for i in range(NSTAGE):
                st = consts.tile([2, BL], F16, tag=f"stage{i}")
                nc.vector.memset(st, 1.0)
                stages.append(st)

            misc_sb = consts.tile([1, 4], F32, tag="misc_sb")
            nc.vector.memset(misc_sb, 0.0)
            c_eps5 = consts.tile([1, 1], F32, tag="c_eps5")
            nc.vector.memset(c_eps5, 1e-5)
            c_eps7 = consts.tile([1, 1], F32, tag="c_eps7")
            nc.vector.memset(c_eps7, 1e-7)

            pre_psum = tc.tile_pool(name="pre_psum", bufs=4, space="PSUM")
            mlv_psum = tc.tile_pool(name="mlv_psum", bufs=1, space="PSUM")
            psum2 = pre_psum.__enter__()
            psum = pre_psum_alias = psum2
            mlvp = mlv_psum.__enter__()
            # ---------- phase 0: XT (per-step scalar inputs) ----------
            # XT[0,:]=0 ; XT[1+k*128+m, :] = (D @ timesT)[k][m]
            for k in range(4):
                p_d = psum2.tile([128, BL], F32, tag="pp")
                nc.tensor.matmul(p_d, DT_sb, timesT_sb[:, k, :],
                                 start=True, stop=(k == 0))
                if k > 0:
                    nc.tensor.matmul(p_d[0:1, :], EV_sb, timesT_sb[:, k - 1, :],
                                     start=False, stop=True)
                xt_sb = work.tile([128, BL], F16, tag="xt_sb")
                nc.scalar.copy(xt_sb, p_d)
                nc.sync.dma_start(out=d_xt[1 + 128 * k:129 + 128 * k, :], in_=xt_sb)
            nc.sync.dma_start(out=d_xt[0:1, :], in_=zrow16)

            # ---------- phase 1: encoder ----------
            p_h0 = psum.tile([128, BL], F32, tag="pp")
            for k in range(6):
                nc.tensor.matmul(p_h0, projT_sb[:, k, :], bertT_sb[:, k, :],
                                 start=(k == 0), stop=(k == 5))
            h0 = work.tile([128, BL], F16, tag="h0")
            nc.scalar.activation(h0, p_h0, AF.Relu, bias=SC_sb[:, 0:1], scale=1.0)
            sq = work.tile([128, BL], F16, tag="sq")
            nc.vector.tensor_tensor(sq, h0, h0, OP.mult)
            p_mu = psum.tile([1, BL], F32, tag="pp")
            nc.tensor.matmul(p_mu, ones_c128, h0, start=True, stop=True)
            p_sq = psum.tile([1, BL], F32, tag="pp")
            nc.tensor.matmul(p_sq, ones_c128, sq, start=True, stop=True)
            mu_n = work.tile([1, BL], F32, tag="mu_n")
            nc.vector.tensor_scalar_mul(mu_n, p_mu, 1.0 / 128)
            msq_n = work.tile([1, BL], F32, tag="msq_n")
            nc.vector.tensor_scalar_mul(msq_n, p_sq, 1.0 / 128)
            var = work.tile([1, BL], F32, tag="var")
            m2 = work.tile([1, BL], F32, tag="m2")
            nc.vector.tensor_tensor(m2, mu_n, mu_n, OP.mult)
            nc.vector.tensor_tensor(var, msq_n, m2, OP.subtract)
            lv_r = work.tile([1, BL], F32, tag="lv_r")
            nc.scalar.activation(lv_r, var, AF.Ln, bias=c_eps5, scale=1.0)
            a16 = work.tile([1, BL], F16, tag="a16")
            nc.scalar.activation(a16, lv_r, AF.Exp, bias=0.0, scale=-0.5)
            c16 = work.tile([1, BL], F16, tag="c16")
            nc.vector.scalar_tensor_tensor(c16, mu_n, -1.0, a16, OP.mult, OP.mult)
            p_a = psum.tile([128, BL], F32, tag="pp")
            nc.tensor.matmul(p_a, ones_r128, a16, start=True, stop=True)
            p_c = psum.tile([128, BL], F32, tag="pp")
            nc.tensor.matmul(p_c, ones_r128, c16, start=True, stop=True)
            u1 = work.tile([128, BL], F16, tag="u1")
            nc.vector.scalar_tensor_tensor(u1, h0, 0.0, p_a, OP.bypass, OP.mult)
            hn = work.tile([128, BL], F16, tag="hn")
            nc.vector.scalar_tensor_tensor(hn, u1, SC_sb[:, 1:2], p_c, OP.mult, OP.add)

            # ---------- phase 2: mean/logvar, z, kl ----------
            p_mlv = mlvp.tile([128, BL], F32, tag="mlv")
            nc.tensor.matmul(p_mlv, mlvT_sb, hn, start=True, stop=True)
            sd = work.tile([ZD, BL], F32, tag="sd")
            nc.scalar.activation(sd, p_mlv[ZD:2 * ZD, :], AF.Exp,
                                 bias=SC_sb[0:ZD, 5:6], scale=0.5)
            mf = work.tile([ZD, BL], F32, tag="mf")
            nc.vector.tensor_scalar_add(mf, p_mlv[0:ZD, :], SC_sb[0:ZD, 7:8])
            e1 = work.tile([ZD, BL], F32, tag="e1")
            nc.vector.tensor_tensor(e1, sd, epsT_sb, OP.mult)
            sd2 = work.tile([ZD, BL], F32, tag="sd2")
            nc.vector.tensor_tensor(sd2, sd, sd, OP.mult)
            t1k = work.tile([ZD, BL], F32, tag="t1k")
            nc.vector.scalar_tensor_tensor(t1k, p_mlv[ZD:2 * ZD, :], SC_sb[0:ZD, 6:7],
                                           sd2, OP.add, OP.subtract)
            u2k = work.tile([ZD, BL], F32, tag="u2k")
            nc.vector.tensor_tensor(u2k, mf, mf, OP.mult)
            kli = work.tile([ZD, BL], F32, tag="kli")
            nc.vector.tensor_tensor(kli, t1k, u2k, OP.subtract)
            p_kl = psum.tile([1, BL], F32, tag="pp")
            nc.tensor.matmul(p_kl, ones_c64, kli, start=True, stop=True)
            nc.vector.tensor_reduce(misc_sb[0:1, 0:1], p_kl, mybir.AxisListType.X,
                                    OP.add)
            # z accumulation: p_mlv[0:64] becomes z
            nc.tensor.matmul(p_mlv[0:ZD, :], I64_sb, e1, start=False, stop=False)
            nc.tensor.matmul(p_mlv[0:ZD, :], MB_sb, ones_r512, start=False, stop=True)

            # ---------- phase 3: planar flows ----------
            G = consts.tile([1, NF * BL], F32, tag="G")
            for i in range(NF):
                z_sb = gwork.tile([ZD, BL], F32, tag="z_sb")
                nc.scalar.copy(z_sb, p_mlv[0:ZD, :])
                p_lin = psum2.tile([1, BL], F32, tag="pp")
                nc.tensor.matmul(p_lin, FW_sb[:, i:i + 1], z_sb, start=True, stop=True)
                th = gwork.tile([1, BL], F32, tag="th")
                nc.scalar.activation(th, p_lin, AF.Tanh, bias=FSC_sb[0:1, i:i + 1],
                                     scale=1.0)
                nc.tensor.matmul(p_mlv[0:ZD, :], UH_sb[0:1, ZD * i:ZD * (i + 1)], th,
                                 start=False, stop=(i == NF - 1))
                s1 = gwork.tile([1, BL], F32, tag="s1")
                nc.vector.scalar_tensor_tensor(s1, th, FSC_sb[0:1, NF + i:NF + i + 1],
                                               th, OP.mult, OP.mult)
                nc.vector.tensor_scalar_add(G[0:1, BL * i:BL * (i + 1)], s1,
                                            FSC_sb[0:1, 2 * NF + i:2 * NF + i + 1])
            G2 = consts.tile([1, NF * BL], F32, tag="G2")
            nc.vector.scalar_tensor_tensor(G2, G, -1.0, G, OP.mult, OP.max)
            ldet = consts.tile([1, NF * BL], F32, tag="ldet")
            nc.scalar.activation(ldet, G2, AF.Ln, bias=c_eps7, scale=1.0)
            nc.vector.tensor_reduce(misc_sb[0:1, 1:2], ldet, mybir.AxisListType.X,
                                    OP.add)
            nc.sync.dma_start(out=d_misc[:, :], in_=misc_sb)

            # decoder init state
            z_fin = consts.tile([ZD, BL], F32, tag="z_fin")
            nc.scalar.copy(z_fin, p_mlv[0:ZD, :])
            p_hd = psum.tile([128, BL], F32, tag="pp")
            nc.tensor.matmul(p_hd, decT_sb, z_fin, start=True, stop=True)
            hgru = consts.tile([128, BL], F16, tag="hgru")
            nc.scalar.activation(hgru, p_hd, AF.Tanh, bias=SC_sb[:, 4:5], scale=1.0)

            mlv_psum.__exit__(None, None, None)
            pre_psum.__exit__(None, None, None)
            gru_psum_cm = tc.tile_pool(name="gru_psum", bufs=1, space="PSUM")
            psumg = gru_psum_cm.__enter__()

            # ---------- phase 4: GRU scan (fully unrolled) ----------
            # per-stream (N=256) matmul groups + interleaved engine emission
            # so the two batch streams pipeline across PE/ACT/DVE.
            p_rz = [psumg.tile([128, BL], F32, tag=f"p_rz{s}", name=f"p_rz{s}")
                    for s in range(2)]
            p_nx = [psumg.tile([128, BL], F32, tag=f"p_nx{s}", name=f"p_nx{s}")
                    for s in range(2)]
            p_out = [psumg.tile([2, 2 * HB], F32, tag=f"p_out{s}", name=f"p_out{s}")
                     for s in range(2)]

            hpool = [h0pool, h1pool]
            spool = [sw0pool, sw1pool]
            h_prev = [hgru[:, 0:HB], hgru[:, HB:BL]]
            for t in range(steps):
                st = stages[t % NSTAGE]
                nc.sync.dma_start(out=st[0:1, :], in_=d_xt[t:t + 1, :])
                cs = [(s * HB, (s + 1) * HB) for s in range(2)]
                # PE: gate matmuls, stream A then B
                for s in range(2):
                    c0, c1 = cs[s]
                    hp = h_prev[s]
                    nc.tensor.matmul(p_rz[s][:, 0:HB], WrT_sb, hp,
                                     start=True, stop=False)
                    nc.tensor.matmul(p_rz[s][:, 0:HB], L2r_sb, st[0:2, c0:c1],
                                     start=False, stop=True)
                    nc.tensor.matmul(p_rz[s][:, HB:BL], WzT_sb, hp,
                                     start=True, stop=False)
                    nc.tensor.matmul(p_rz[s][:, HB:BL], L2z_sb, st[0:2, c0:c1],
                                     start=False, stop=True)
                    nc.tensor.matmul(p_nx[s][:, 0:HB], WnT_sb, hp,
                                     start=True, stop=True)
                    nc.tensor.matmul(p_nx[s][:, HB:BL], winr_sb, st[0:1, c0:c1],
                                     start=True, stop=True)
                # ACT: sigmoids
                rz = [spool[s].tile([128, BL], F16, tag="rz", name="rz")
                      for s in range(2)]
                for s in range(2):
                    nc.scalar.activation(rz[s], p_rz[s], AF.Sigmoid)
                # DVE: n-gate chain
                q = [spool[s].tile([128, HB], F16, tag="q", name="q")
                     for s in range(2)]
                t2 = [spool[s].tile([128, HB], F16, tag="t2", name="t2")
                      for s in range(2)]
                for s in range(2):
                    nc.vector.scalar_tensor_tensor(q[s], p_nx[s][:, 0:HB],
                                                   SC_sb[:, 2:3],
                                                   rz[s][:, 0:HB], OP.add, OP.mult)
                    nc.vector.tensor_tensor(t2[s], q[s], p_nx[s][:, HB:BL], OP.add)
                # ACT: tanh
                ng = [spool[s].tile([128, HB], F16, tag="ng", name="ng")
                      for s in range(2)]
                for s in range(2):
                    nc.scalar.activation(ng[s], t2[s], AF.Tanh, bias=SC_sb[:, 3:4],
                                         scale=1.0)
                # DVE: blend, stream A fully first so PE can restart stream A
                h_new = [hpool[s].tile([128, HB], F16, tag="h", name="h")
                         for s in range(2)]
                for s in range(2):
                    d_ = spool[s].tile([128, HB], F16, tag="d", name="d")
                    nc.vector.tensor_tensor(d_, h_prev[s], ng[s], OP.subtract)
                    e_ = spool[s].tile([128, HB], F16, tag="e", name="e")
                    nc.vector.tensor_tensor(e_, rz[s][:, HB:BL], d_, OP.mult)
                    nc.vector.tensor_tensor(h_new[s], ng[s], e_, OP.add)
                # PE: head matmuls; evac every 2 steps
                sl = t % 2
                for s in range(2):
                    c0, c1 = cs[s]
                    nc.tensor.matmul(p_out[s][:, HB * sl:HB * (sl + 1)], headT_sb,
                                     h_new[s], start=True, stop=True)
                    if sl == 1:
                        ob = spool[s].tile([2, 2 * HB], F32, tag="ob", name="ob")
                        if s == 0:
                            nc.scalar.copy(ob, p_out[s])
                        else:
                            nc.vector.tensor_copy(ob, p_out[s])
                        nc.sync.dma_start(
                            out=d_outs[t - 1:t + 1, :, c0:c1].rearrange(
                                "u c b -> c u b"),
                            in_=ob.rearrange("c (u b) -> c u b", u=2))
                    elif t == steps - 1:
                        ob = spool[s].tile([2, HB], F32, tag="ob2", name="ob2")
                        if s == 0:
                            nc.scalar.copy(ob, p_out[s][:, 0:HB])
                        else:
                            nc.vector.tensor_copy(ob, p_out[s][:, 0:HB])
                        nc.sync.dma_start(out=d_outs[t, :, c0:c1], in_=ob)
                    h_prev[s] = h_new[s]
            gru_psum_cm.__exit__(None, None, None)

    nc.compile()
    return nc


def _prep_inputs(inputs):
    """Host-side: shard batch across cores, transpose to feature-major
    layouts, pack parameters (replicated)."""
    x = {k: np.asarray(v) for k, v in inputs.items()}
    f32 = np.float32
    f16 = np.float16

    bertT = np.ascontiguousarray(x["bert_emb"].astype(f32).T).astype(f16)  # [768,B]
    timesT = np.ascontiguousarray(x["times"].astype(f32).T)                # [T,B]
    epsT = np.ascontiguousarray(x["eps"].astype(f32).T)                    # [64,B]

    proj_w = x["proj_w"].astype(f32)
    projT = np.ascontiguousarray(proj_w.T).astype(f16)                     # [768,128]
    mean_w = x["mean_w"].astype(f32)
    logv_w = x["logv_w"].astype(f32)
    mlvT = np.ascontiguousarray(np.concatenate([mean_w, logv_w], 0).T).astype(f16)
    decT = np.ascontiguousarray(x["dec_init_w"].astype(f32).T)             # [64,128]

    w_hh = x["gru_w_hh"].astype(f32)
    w_ih = x["gru_w_ih"].astype(f32)[:, 0]
    b_ih = x["gru_b_ih"].astype(f32)
    b_hh = x["gru_b_hh"].astype(f32)
    WrT = np.ascontiguousarray(w_hh[0:128].T).astype(f16)
    WzT = np.ascontiguousarray(w_hh[128:256].T).astype(f16)
    WnT = np.ascontiguousarray(w_hh[256:384].T).astype(f16)
    L2r = np.stack([w_ih[0:128], b_ih[0:128] + b_hh[0:128]]).astype(f16)   # [2,128]
    L2z = np.stack([w_ih[128:256], b_ih[128:256] + b_hh[128:256]]).astype(f16)
    winr = w_ih[256:384][None, :].astype(f16)                              # [1,128]
    headT = np.ascontiguousarray(x["head_w"].astype(f32).T).astype(f16)    # [128,2]

    D = np.eye(128, dtype=f32) - np.eye(128, k=-1, dtype=f32)
    DT = np.ascontiguousarray(D.T)
    EV = np.zeros((128, 1), f32)
    EV[127, 0] = -1.0

    mean_b_eff = x["mean_b"].astype(f32) + mean_w @ x["ln_b"].astype(f32)
    logv_b_eff = x["logv_b"].astype(f32) + logv_w @ x["ln_b"].astype(f32)
    SC = np.zeros((128, 8), f32)
    SC[:, 0] = x["proj_b"].astype(f32)
    SC[:, 1] = x["ln_g"].astype(f32)
    SC[:, 2] = b_hh[256:384]
    SC[:, 3] = b_ih[256:384]
    SC[:, 4] = x["dec_init_b"].astype(f32)
    SC[0:ZD, 5] = 0.5 * logv_b_eff
    SC[0:ZD, 6] = logv_b_eff + 1.0
    SC[0:ZD, 7] = mean_b_eff

    nf_w = x["nf_w"].astype(f32)
    nf_u = x["nf_u"].astype(f32)
    nf_b = x["nf_b"].astype(f32)
    FW = np.ascontiguousarray(nf_w.T)                                      # [64,8]
    UH = np.zeros((1, NF * ZD), f32)
    FSC = np.zeros((1, 3 * NF), f32)
    for i in range(NF):
        w, u = nf_w[i], nf_u[i]
        wu = float(np.sum(w * u))
        sp = float(np.log1p(np.exp(-abs(wu))) + max(wu, 0.0))  # softplus stable
        u_hat = (sp - 1.0 - wu) * (w / np.linalg.norm(w)) + u
        swu = float(np.sum(w * u_hat))
        UH[0, ZD * i:ZD * (i + 1)] = u_hat
        FSC[0, i] = nf_b[i]
        FSC[0, NF + i] = -swu
        FSC[0, 2 * NF + i] = 1.0 + swu

    MB = mean_b_eff[None, :].astype(f32)
    I64 = np.eye(ZD, dtype=f32)

    shared = dict(projT=projT, mlvT=mlvT, decT=decT, WrT=WrT, WzT=WzT, WnT=WnT,
                  L2r=L2r, L2z=L2z, winr=winr, headT=headT, DT=DT, EV=EV, SC=SC,
                  FW=FW, UH=UH, MB=MB, FSC=FSC, I64=I64)
    in_maps = []
    for c in range(NC_):
        sl = slice(c * BL, (c + 1) * BL)
        m = dict(shared)
        m["bertT"] = np.ascontiguousarray(bertT[:, sl])
        m["timesT"] = np.ascontiguousarray(timesT[:, sl])
        m["epsT"] = np.ascontiguousarray(epsT[:, sl])
        in_maps.append(m)
    return in_maps


_PROGRAM = None


def _get_program():
    global _PROGRAM
    if _PROGRAM is None:
        _PROGRAM = build_program()
    return _PROGRAM


def run_on_device(inputs, trace=False, **kw):
    nc = _get_program()
    in_maps = _prep_inputs(inputs)
    res = run_bass_kernel_spmd(nc, in_maps, core_ids=list(range(NC_)),
                               trace=trace, **kw)
    return res


def _assemble(results, inputs):
    head_b = np.asarray(inputs["head_b"], np.float32)
    pred = np.empty((B, STEPS), np.float32)
    stop = np.empty((B, STEPS), np.float32)
    kl_sum = 0.0
    ld_sum = 0.0
    for c, r in enumerate(results):
        o = r["outs"]  # [513, 2, BL]
        sl = slice(c * BL, (c + 1) * BL)
        pred[sl] = o[:, 0, :].T
        stop[sl] = o[:, 1, :].T
        kl_sum += float(r["misc"][0, 0])
        ld_sum += float(r["misc"][0, 1])
    pred += head_b[0]
    stop += head_b[1]
    kl = np.float32(-0.5 * kl_sum / (B * ZD))
    nf_loss = np.float32(-ld_sum / B)
    return pred, stop, np.asarray(kl), np.asarray(nf_loss)


def kernel(**inputs):
    res = run_on_device(inputs, trace=False)
    return _assemble(res.results, inputs)


# revision 24
# speedup vs baseline: 1.1987x; 1.1987x over previous
"""Trainium2 Bass kernel for nn_CasFlowSeqModel (VAE encoder + planar flows +
autoregressive GRU decoder).

Sharding: pure data-parallel over batch B=4096 across 8 NeuronCores
(512 rows/core), params replicated. On-chip layout is "transposed"
(feature-on-partitions, batch-on-free-dim) so the GRU recurrence needs no
transposes: state hT [RU=128 partitions, batch free] feeds matmuls directly.
GRU runs 513 fully-unrolled steps, 2 interleaved batch streams of 256 columns
to overlap PE / ACT(sigmoid+tanh) / DVE work. fp16 operands, fp32 PSUM.
"""

import sys

import numpy as np

sys.path.insert(0, "/opt/trn_rl_repo")

import concourse.bass as bass  # noqa: E402
from concourse import bacc  # noqa: E402
import concourse.mybir as mybir  # noqa: E402
import concourse.tile as tile  # noqa: E402
from concourse.bass_utils import run_bass_kernel_spmd  # noqa: E402
from concourse import bass_utils as _bu  # noqa: E402

# Re-enable walrus LDWEIGHTS scheduling opt (hardcoded off in bass_utils):
# overlaps weight loads with in-flight matmuls on the PE.
if not getattr(_bu, "_ldw_patched", False):
    _orig_run_command = _bu.run_command

    def _run_command_ldw(argv, **kw):
        argv = list(argv)
        return _orig_run_command(argv, **kw)

    _bu.run_command = _run_command_ldw
    _bu._ldw_patched = True

F16 = mybir.dt.float16
F32 = mybir.dt.float32
AF = mybir.ActivationFunctionType
OP = mybir.AluOpType

B, T = 4096, 512
NC_ = 8
BL = B // NC_          # 512 batch rows per core
HB = BL // 2           # 256 per stream
STEPS = T + 1          # 513
RU = 128
EMB = 128
ZD = 64
NF = 8
BERT = 768


def build_program(steps=STEPS):
    nc = bacc.Bacc("TRN2", target_bir_lowering=False)

    # ---- DRAM I/O (per core; names keyed in in_maps) ----
    d_bertT = nc.dram_tensor("bertT", [BERT, BL], F16, kind="ExternalInput")
    d_timesT = nc.dram_tensor("timesT", [T, BL], F32, kind="ExternalInput")
    d_epsT = nc.dram_tensor("epsT", [ZD, BL], F32, kind="ExternalInput")
    d_projT = nc.dram_tensor("projT", [BERT, EMB], F16, kind="ExternalInput")
    d_mlvT = nc.dram_tensor("mlvT", [EMB, 2 * ZD], F16, kind="ExternalInput")
    d_decT = nc.dram_tensor("decT", [ZD, RU], F32, kind="ExternalInput")
    d_WrT = nc.dram_tensor("WrT", [RU, RU], F16, kind="ExternalInput")
    d_WzT = nc.dram_tensor("WzT", [RU, RU], F16, kind="ExternalInput")
    d_WnT = nc.dram_tensor("WnT", [RU, RU], F16, kind="ExternalInput")
    d_L2r = nc.dram_tensor("L2r", [2, RU], F16, kind="ExternalInput")
    d_L2z = nc.dram_tensor("L2z", [2, RU], F16, kind="ExternalInput")
    d_winr = nc.dram_tensor("winr", [1, RU], F16, kind="ExternalInput")
    d_headT = nc.dram_tensor("headT", [RU, 2], F16, kind="ExternalInput")
    d_DT = nc.dram_tensor("DT", [128, 128], F32, kind="ExternalInput")
    d_EV = nc.dram_tensor("EV", [128, 1], F32, kind="ExternalInput")
    d_SC = nc.dram_tensor("SC", [128, 8], F32, kind="ExternalInput")
    d_FW = nc.dram_tensor("FW", [ZD, NF], F32, kind="ExternalInput")
    d_UH = nc.dram_tensor("UH", [1, NF * ZD], F32, kind="ExternalInput")
    d_MB = nc.dram_tensor("MB", [1, ZD], F32, kind="ExternalInput")
    d_FSC = nc.dram_tensor("FSC", [1, 3 * NF], F32, kind="ExternalInput")
    d_I64 = nc.dram_tensor("I64", [ZD, ZD], F32, kind="ExternalInput")

    d_outs = nc.dram_tensor("outs", [STEPS, 2, BL], F32, kind="ExternalOutput")
    d_misc = nc.dram_tensor("misc", [1, 4], F32, kind="ExternalOutput")

    with tile.TileContext(nc) as tc:
        with (
            tc.tile_pool(name="consts", bufs=1) as consts,
            tc.tile_pool(name="work", bufs=2) as work,
            tc.tile_pool(name="gwork", bufs=2) as gwork,
            tc.tile_pool(name="dram", bufs=1, space="DRAM") as dpool,
            tc.tile_pool(name="h0p", bufs=2) as h0pool,
            tc.tile_pool(name="h1p", bufs=2) as h1pool,
            tc.tile_pool(name="sw0", bufs=2) as sw0pool,
            tc.tile_pool(name="sw1", bufs=2) as sw1pool,
        ):
            d_xt = dpool.tile([STEPS, BL], F16, tag="xt")
            # ---------- constants to SBUF ----------
            def cload(dram_ap, shape, dtype, tag):
                t_ = consts.tile(shape, dtype, tag=tag)
                nc.sync.dma_start(out=t_, in_=dram_ap)
                return t_

            bertT_sb = cload(d_bertT.rearrange("(c p) b -> p c b", p=128),
                             [128, 6, BL], F16, "bertT")
            timesT_sb = cload(d_timesT.rearrange("(c p) b -> p c b", p=128),
                              [128, 4, BL], F32, "timesT")
            epsT_sb = cload(d_epsT[:, :], [ZD, BL], F32, "epsT")
            projT_sb = cload(d_projT.rearrange("(c p) m -> p c m", p=128),
                             [128, 6, EMB], F16, "projT")
            mlvT_sb = cload(d_mlvT[:, :], [EMB, 2 * ZD], F16, "mlvT")
            decT_sb = cload(d_decT[:, :], [ZD, RU], F32, "decT")
            WrT_sb = cload(d_WrT[:, :], [RU, RU], F16, "WrT")
            WzT_sb = cload(d_WzT[:, :], [RU, RU], F16, "WzT")
            WnT_sb = cload(d_WnT[:, :], [RU, RU], F16, "WnT")
            L2r_sb = cload(d_L2r[:, :], [2, RU], F16, "L2r")
            L2z_sb = cload(d_L2z[:, :], [2, RU], F16, "L2z")
            winr_sb = cload(d_winr[:, :], [1, RU], F16, "winr")
            headT_sb = cload(d_headT[:, :], [RU, 2], F16, "headT")
            DT_sb = cload(d_DT[:, :], [128, 128], F32, "DT")
            EV_sb = cload(d_EV[:, :], [128, 1], F32, "EV")
            SC_sb = cload(d_SC[:, :], [128, 8], F32, "SC")
            FW_sb = cload(d_FW[:, :], [ZD, NF], F32, "FW")
            UH_sb = cload(d_UH[:, :], [1, NF * ZD], F32, "UH")
            MB_sb = cload(d_MB[:, :], [1, ZD], F32, "MB")
            FSC_sb = cload(d_FSC[:, :], [1, 3 * NF], F32, "FSC")
            I64_sb = cload(d_I64[:, :], [ZD, ZD], F32, "I64")

            ones_c128 = consts.tile([128, 1], F16, tag="ones_c128")
            nc.vector.memset(ones_c128, 1.0)
            ones_c64 = consts.tile([ZD, 1], F32, tag="ones_c64")
            nc.vector.memset(ones_c64, 1.0)
            ones_r128 = consts.tile([1, 128], F16, tag="ones_r128")
            nc.vector.memset(ones_r128, 1.0)
            ones_r512 = consts.tile([1, BL], F32, tag="ones_r512")
            nc.vector.memset(ones_r512, 1.0)
            zrow16 = consts.tile([1, BL], F16, tag="zrow16")
            nc.vector.memset(zrow16, 0.0)
            # rotating x-staging tiles: row0 <- x_t via DMA, row1 = ones
            NSTAGE = 8
            stages = []
            for i in range(NSTAGE):
                st = consts.tile([2, BL], F16, tag=f"stage{i}")
                nc.vector.memset(st, 1.0)
                stages.append(st)

            misc_sb = consts.tile([1, 4], F32, tag="misc_sb")
            nc.vector.memset(misc_sb, 0.0)
            c_eps5 = consts.tile([1, 1], F32, tag="c_eps5")
            nc.vector.memset(c_eps5, 1e-5)
            c_eps7 = consts.tile([1, 1], F32, tag="c_eps7")
            nc.vector.memset(c_eps7, 1e-7)

            pre_psum = tc.tile_pool(name="pre_psum", bufs=4, space="PSUM")
            mlv_psum = tc.tile_pool(name="mlv_psum", bufs=1, space="PSUM")
            psum2 = pre_psum.__enter__()
            psum = pre_psum_alias = psum2
            mlvp = mlv_psum.__enter__()
            # ---------- phase 0: XT (per-step scalar inputs) ----------
            # XT[0,:]=0 ; XT[1+k*128+m, :] = (D @ timesT)[k][m]
            for k in range(4):
                p_d = psum2.tile([128, BL], F32, tag="pp")
                nc.tensor.matmul(p_d, DT_sb, timesT_sb[:, k, :],
                                 start=True, stop=(k == 0))
                if k > 0:
                    nc.tensor.matmul(p_d[0:1, :], EV_sb, timesT_sb[:, k - 1, :],
                                     start=False, stop=True)
                xt_sb = work.tile([128, BL], F16, tag="xt_sb")
                nc.scalar.copy(xt_sb, p_d)
                nc.sync.dma_start(out=d_xt[1 + 128 * k:129 + 128 * k, :], in_=xt_sb)
            nc.sync.dma_start(out=d_xt[0:1, :], in_=zrow16)

            # ---------- phase 1: encoder ----------
            p_h0 = psum.tile([128, BL], F32, tag="pp")
            for k in range(6):
                nc.tensor.matmul(p_h0, projT_sb[:, k, :], bertT_sb[:, k, :],
                                 start=(k == 0), stop=(k == 5))
            h0 = work.tile([128, BL], F16, tag="h0")
            nc.scalar.activation(h0, p_h0, AF.Relu, bias=SC_sb[:, 0:1], scale=1.0)
            sq = work.tile([128, BL], F16, tag="sq")
            nc.vector.tensor_tensor(sq, h0, h0, OP.mult)
            p_mu = psum.tile([1, BL], F32, tag="pp")
            nc.tensor.matmul(p_mu, ones_c128, h0, start=True, stop=True)
            p_sq = psum.tile([1, BL], F32, tag="pp")
            nc.tensor.matmul(p_sq, ones_c128, sq, start=True, stop=True)
            mu_n = work.tile([1, BL], F32, tag="mu_n")
            nc.vector.tensor_scalar_mul(mu_n, p_mu, 1.0 / 128)
            msq_n = work.tile([1, BL], F32, tag="msq_n")
            nc.vector.tensor_scalar_mul(msq_n, p_sq, 1.0 / 128)
            var = work.tile([1, BL], F32, tag="var")
            m2 = work.tile([1, BL], F32, tag="m2")
            nc.vector.tensor_tensor(m2, mu_n, mu_n, OP.mult)
            nc.vector.tensor_tensor(var, msq_n, m2, OP.subtract)
            lv_r = work.tile([1, BL], F32, tag="lv_r")
            nc.scalar.activation(lv_r, var, AF.Ln, bias=c_eps5, scale=1.0)
            a16 = work.tile([1, BL], F16, tag="a16")
            nc.scalar.activation(a16, lv_r, AF.Exp, bias=0.0, scale=-0.5)
            c16 = work.tile([1, BL], F16, tag="c16")
            nc.vector.scalar_tensor_tensor(c16, mu_n, -1.0, a16, OP.mult, OP.mult)
            p_a = psum.tile([128, BL], F32, tag="pp")
            nc.tensor.matmul(p_a, ones_r128, a16, start=True, stop=True)
            p_c = psum.tile([128, BL], F32, tag="pp")
            nc.tensor.matmul(p_c, ones_r128, c16, start=True, stop=True)
            u1 = work.tile([128, BL], F16, tag="u1")
            nc.vector.scalar_tensor_tensor(u1, h0, 0.0, p_a, OP.bypass, OP.mult)
            hn = work.tile([128, BL], F16, tag="hn")
            nc.vector.scalar_tensor_tensor(hn, u1, SC_sb[:, 1:2], p_c, OP.mult, OP.add)

            # ---------- phase 2: mean/logvar, z, kl ----------
            p_mlv = mlvp.tile([128, BL], F32, tag="mlv")
            nc.tensor.matmul(p_mlv, mlvT_sb, hn, start=True, stop=True)
            sd = work.tile([ZD, BL], F32, tag="sd")
            nc.scalar.activation(sd, p_mlv[ZD:2 * ZD, :], AF.Exp,
                                 bias=SC_sb[0:ZD, 5:6], scale=0.5)
            mf = work.tile([ZD, BL], F32, tag="mf")
            nc.vector.tensor_scalar_add(mf, p_mlv[0:ZD, :], SC_sb[0:ZD, 7:8])
            e1 = work.tile([ZD, BL], F32, tag="e1")
            nc.vector.tensor_tensor(e1, sd, epsT_sb, OP.mult)
            sd2 = work.tile([ZD, BL], F32, tag="sd2")
            nc.vector.tensor_tensor(sd2, sd, sd, OP.mult)
            t1k = work.tile([ZD, BL], F32, tag="t1k")
            nc.vector.scalar_tensor_tensor(t1k, p_mlv[ZD:2 * ZD, :], SC_sb[0:ZD, 6:7],
                                           sd2, OP.add, OP.subtract)
            u2k = work.tile([ZD, BL], F32, tag="u2k")
            nc.vector.tensor_tensor(u2k, mf, mf, OP.mult)
            kli = work.tile([ZD, BL], F32, tag="kli")
            nc.vector.tensor_tensor(kli, t1k, u2k, OP.subtract)
            p_kl = psum.tile([1, BL], F32, tag="pp")
            nc.tensor.matmul(p_kl, ones_c64, kli, start=True, stop=True)
            nc.vector.tensor_reduce(misc_sb[0:1, 0:1], p_kl, mybir.AxisListType.X,
                                    OP.add)
            # z accumulation: p_mlv[0:64] becomes z
            nc.tensor.matmul(p_mlv[0:ZD, :], I64_sb, e1, start=False, stop=False)
            nc.tensor.matmul(p_mlv[0:ZD, :], MB_sb, ones_r512, start=False, stop=True)

            # ---------- phase 3: planar flows ----------
            G = consts.tile([1, NF * BL], F32, tag="G")
            for i in range(NF):
                z_sb = gwork.tile([ZD, BL], F32, tag="z_sb")
                nc.scalar.copy(z_sb, p_mlv[0:ZD, :])
                p_lin = psum2.tile([1, BL], F32, tag="pp")
                nc.tensor.matmul(p_lin, FW_sb[:, i:i + 1], z_sb, start=True, stop=True)
                th = gwork.tile([1, BL], F32, tag="th")
                nc.scalar.activation(th, p_lin, AF.Tanh, bias=FSC_sb[0:1, i:i + 1],
                                     scale=1.0)
                nc.tensor.matmul(p_mlv[0:ZD, :], UH_sb[0:1, ZD * i:ZD * (i + 1)], th,
                                 start=False, stop=(i == NF - 1))
                s1 = gwork.tile([1, BL], F32, tag="s1")
                nc.vector.scalar_tensor_tensor(s1, th, FSC_sb[0:1, NF + i:NF + i + 1],
                                               th, OP.mult, OP.mult)
                nc.vector.tensor_scalar_add(G[0:1, BL * i:BL * (i + 1)], s1,
                                            FSC_sb[0:1, 2 * NF + i:2 * NF + i + 1])
            G2 = consts.tile([1, NF * BL], F32, tag="G2")
            nc.vector.scalar_tensor_tensor(G2, G, -1.0, G, OP.mult, OP.max)
            ldet = consts.tile([1, NF * BL], F32, tag="ldet")
            nc.scalar.activation(ldet, G2, AF.Ln, bias=c_eps7, scale=1.0)
            nc.vector.tensor_reduce(misc_sb[0:1, 1:2], ldet, mybir.AxisListType.X,
                                    OP.add)
            nc.sync.dma_start(out=d_misc[:, :], in_=misc_sb)

            # decoder init state
            z_fin = consts.tile([ZD, BL], F32, tag="z_fin")
            nc.scalar.copy(z_fin, p_mlv[0:ZD, :])
            p_hd = psum.tile([128, BL], F32, tag="pp")
            nc.tensor.matmul(p_hd, decT_sb, z_fin, start=True, stop=True)
            hgru = consts.tile([128, BL], F16, tag="hgru")
            nc.scalar.activation(hgru, p_hd, AF.Tanh, bias=SC_sb[:, 4:5], scale=1.0)

            mlv_psum.__exit__(None, None, None)
            pre_psum.__exit__(None, None, None)
            gru_psum_cm = tc.tile_pool(name="gru_psum", bufs=1, space="PSUM")
            psumg = gru_psum_cm.__enter__()

            # ---------- phase 4: GRU scan (fully unrolled) ----------
            # per-stream (N=256) matmul groups + interleaved engine emission
            # so the two batch streams pipeline across PE/ACT/DVE.
            p_rz = [psumg.tile([128, BL], F32, tag=f"p_rz{s}", name=f"p_rz{s}")
                    for s in range(2)]
            p_nx = [psumg.tile([128, BL], F32, tag=f"p_nx{s}", name=f"p_nx{s}")
                    for s in range(2)]
            p_out = [psumg.tile([2, 2 * HB], F32, tag=f"p_out{s}", name=f"p_out{s}")
                     for s in range(2)]

            hpool = [h0pool, h1pool]
            spool = [sw0pool, sw1pool]
            h_prev = [hgru[:, 0:HB], hgru[:, HB:BL]]
            for t in range(steps):
                st = stages[t % NSTAGE]
                nc.sync.dma_start(out=st[0:1, :], in_=d_xt[t:t + 1, :])
                cs = [(s * HB, (s + 1) * HB) for s in range(2)]
                # PE: gate matmuls, stream A then B
                for s in range(2):
                    c0, c1 = cs[s]
                    hp = h_prev[s]
                    nc.tensor.matmul(p_rz[s][:, 0:HB], WrT_sb, hp,
                                     start=True, stop=False)
                    nc.tensor.matmul(p_rz[s][:, 0:HB], L2r_sb, st[0:2, c0:c1],
                                     start=False, stop=True)
                    nc.tensor.matmul(p_rz[s][:, HB:BL], WzT_sb, hp,
                                     start=True, stop=False)
                    nc.tensor.matmul(p_rz[s][:, HB:BL], L2z_sb, st[0:2, c0:c1],
                                     start=False, stop=True)
                    nc.tensor.matmul(p_nx[s][:, 0:HB], WnT_sb, hp,
                                     start=True, stop=True)
                    nc.tensor.matmul(p_nx[s][:, HB:BL], winr_sb, st[0:1, c0:c1],
                                     start=True, stop=True)
                # ACT: sigmoids
                rz = [spool[s].tile([128, BL], F16, tag="rz", name="rz")
                      for s in range(2)]
                for s in range(2):
                    nc.scalar.activation(rz[s], p_rz[s], AF.Sigmoid)
                # DVE: n-gate chain
                q = [spool[s].tile([128, HB], F16, tag="q", name="q")
                     for s in range(2)]
                t2 = [spool[s].tile([128, HB], F16, tag="t2", name="t2")
                      for s in range(2)]
                for s in range(2):
                    nc.vector.scalar_tensor_tensor(q[s], p_nx[s][:, 0:HB],
                                                   SC_sb[:, 2:3],
                                                   rz[s][:, 0:HB], OP.add, OP.mult)
                    nc.vector.tensor_tensor(t2[s], q[s], p_nx[s][:, HB:BL], OP.add)
                # ACT: tanh
                ng = [spool[s].tile([128, HB], F16, tag="ng", name="ng")
                      for s in range(2)]
                for s in range(2):
                    nc.scalar.activation(ng[s], t2[s], AF.Tanh, bias=SC_sb[:, 3:4],
                                         scale=1.0)
                # DVE: blend, stream A fully first so PE can restart stream A
                h_new = [hpool[s].tile([128, HB], F16, tag="h", name="h")
                         for s in range(2)]
                for s in range(2):
                    d_ = spool[s].tile([128, HB], F16, tag="d", name="d")
                    nc.vector.tensor_tensor(d_, h_prev[s], ng[s], OP.subtract)
                    e_ = spool[s].tile([128, HB], F16, tag="e", name="e")
                    nc.vector.tensor_tensor(e_, rz[s][:, HB:BL], d_, OP.mult)
                    nc.vector.tensor_tensor(h_new[s], ng[s], e_, OP.add)
                # PE: head matmuls; evac every 2 steps
                sl = t % 2
                for s in range(2):
                    c0, c1 = cs[s]
                    nc.tensor.matmul(p_out[s][:, HB * sl:HB * (sl + 1)], headT_sb,
                                     h_new[s], start=True, stop=True)
                    if sl == 1:
                        ob = spool[s].tile([2, 2 * HB], F32, tag="ob", name="ob")
                        if s == 0:
                            nc.scalar.copy(ob, p_out[s])
                        else:
                            nc.vector.tensor_copy(ob, p_out[s])
                        nc.sync.dma_start(
                            out=d_outs[t - 1:t + 1, :, c0:c1].rearrange(
                                "u c b -> c u b"),
                            in_=ob.rearrange("c (u b) -> c u b", u=2))
                    elif t == steps - 1:
                        ob = spool[s].tile([2, HB], F32, tag="ob2", name="ob2")
                        if s == 0:
                            nc.scalar.copy(ob, p_out[s][:, 0:HB])
                        else:
                            nc.vector.tensor_copy(ob, p_out[s][:, 0:HB])
                        nc.sync.dma_start(out=d_outs[t, :, c0:c1], in_=ob)
                    h_prev[s] = h_new[s]
            gru_psum_cm.__exit__(None, None, None)

    nc.compile()
    return nc


def _prep_inputs(inputs):
    """Host-side: shard batch across cores, transpose to feature-major
    layouts, pack parameters (replicated)."""
    x = {k: np.asarray(v) for k, v in inputs.items()}
    f32 = np.float32
    f16 = np.float16

    bertT = np.ascontiguousarray(x["bert_emb"].astype(f32).T).astype(f16)  # [768,B]
    timesT = np.ascontiguousarray(x["times"].astype(f32).T)                # [T,B]
    epsT = np.ascontiguousarray(x["eps"].astype(f32).T)                    # [64,B]

    proj_w = x["proj_w"].astype(f32)
    projT = np.ascontiguousarray(proj_w.T).astype(f16)                     # [768,128]
    mean_w = x["mean_w"].astype(f32)
    logv_w = x["logv_w"].astype(f32)
    mlvT = np.ascontiguousarray(np.concatenate([mean_w, logv_w], 0).T).astype(f16)
    decT = np.ascontiguousarray(x["dec_init_w"].astype(f32).T)             # [64,128]

    w_hh = x["gru_w_hh"].astype(f32)
    w_ih = x["gru_w_ih"].astype(f32)[:, 0]
    b_ih = x["gru_b_ih"].astype(f32)
    b_hh = x["gru_b_hh"].astype(f32)
    WrT = np.ascontiguousarray(w_hh[0:128].T).astype(f16)
    WzT = np.ascontiguousarray(w_hh[128:256].T).astype(f16)
    WnT = np.ascontiguousarray(w_hh[256:384].T).astype(f16)
    L2r = np.stack([w_ih[0:128], b_ih[0:128] + b_hh[0:128]]).astype(f16)   # [2,128]
    L2z = np.stack([w_ih[128:256], b_ih[128:256] + b_hh[128:256]]).astype(f16)
    winr = w_ih[256:384][None, :].astype(f16)                              # [1,128]
    headT = np.ascontiguousarray(x["head_w"].astype(f32).T).astype(f16)    # [128,2]

    D = np.eye(128, dtype=f32) - np.eye(128, k=-1, dtype=f32)
    DT = np.ascontiguousarray(D.T)
    EV = np.zeros((128, 1), f32)
    EV[127, 0] = -1.0

    mean_b_eff = x["mean_b"].astype(f32) + mean_w @ x["ln_b"].astype(f32)
    logv_b_eff = x["logv_b"].astype(f32) + logv_w @ x["ln_b"].astype(f32)
    SC = np.zeros((128, 8), f32)
    SC[:, 0] = x["proj_b"].astype(f32)
    SC[:, 1] = x["ln_g"].astype(f32)
    SC[:, 2] = b_hh[256:384]
    SC[:, 3] = b_ih[256:384]
    SC[:, 4] = x["dec_init_b"].astype(f32)
    SC[0:ZD, 5] = 0.5 * logv_b_eff
    SC[0:ZD, 6] = logv_b_eff + 1.0
    SC[0:ZD, 7] = mean_b_eff

    nf_w = x["nf_w"].astype(f32)
    nf_u = x["nf_u"].astype(f32)
    nf_b = x["nf_b"].astype(f32)
    FW = np.ascontiguousarray(nf_w.T)                                      # [64,8]
    UH = np.zeros((1, NF * ZD), f32)
    FSC = np.zeros((1, 3 * NF), f32)
    for i in range(NF):
        w, u = nf_w[i], nf_u[i]
        wu = float(np.sum(w * u))
        sp = float(np.log1p(np.exp(-abs(wu))) + max(wu, 0.0))  # softplus stable
        u_hat = (sp - 1.0 - wu) * (w / np.linalg.norm(w)) + u
        swu = float(np.sum(w * u_hat))
        UH[0, ZD * i:ZD * (i + 1)] = u_hat
        FSC[0, i] = nf_b[i]
        FSC[0, NF + i] = -swu
        FSC[0, 2 * NF + i] = 1.0 + swu

    MB = mean_b_eff[None, :].astype(f32)
    I64 = np.eye(ZD, dtype=f32)

    shared = dict(projT=projT, mlvT=mlvT, decT=decT, WrT=WrT, WzT=WzT, WnT=WnT,
                  L2r=L2r, L2z=L2z, winr=winr, headT=headT, DT=DT, EV=EV, SC=SC,
                  FW=FW, UH=UH, MB=MB, FSC=FSC, I64=I64)
    in_maps = []
    for c in range(NC_):
        sl = slice(c * BL, (c + 1) * BL)
        m = dict(shared)
        m["bertT"] = np.ascontiguousarray(bertT[:, sl])
        m["timesT"] = np.ascontiguousarray(timesT[:, sl])
        m["epsT"] = np.ascontiguousarray(epsT[:, sl])
        in_maps.append(m)
    return in_maps


_PROGRAM = None


def _get_program():
    global _PROGRAM
    if _PROGRAM is None:
        _PROGRAM = build_program()
    return _PROGRAM


def run_on_device(inputs, trace=False, **kw):
    nc = _get_program()
    in_maps = _prep_inputs(inputs)
    res = run_bass_kernel_spmd(nc, in_maps, core_ids=list(range(NC_)),
                               trace=trace, **kw)
    return res


def _assemble(results, inputs):
    head_b = np.asarray(inputs["head_b"], np.float32)
    pred = np.empty((B, STEPS), np.float32)
    stop = np.empty((B, STEPS), np.float32)
    kl_sum = 0.0
    ld_sum = 0.0
    for c, r in enumerate(results):
        o = r["outs"]  # [513, 2, BL]
        sl = slice(c * BL, (c + 1) * BL)
        pred[sl] = o[:, 0, :].T
        stop[sl] = o[:, 1, :].T
        kl_sum += float(r["misc"][0, 0])
        ld_sum += float(r["misc"][0, 1])
    pred += head_b[0]
    stop += head_b[1]
    kl = np.float32(-0.5 * kl_sum / (B * ZD))
    nf_loss = np.float32(-ld_sum / B)
    return pred, stop, np.asarray(kl), np.asarray(nf_loss)


def kernel(**inputs):
    res = run_on_device(inputs, trace=False)
    return _assemble(res.results, inputs)
